# revision 3
# baseline (speedup 1.0000x reference)
"""LocallyConnected1d Trainium2 kernel (bf16).

Problem: out[b, oc, w] = sum_{ic,k} xp[b, ic, w+k] * W[w, oc, ic, k] + bias[oc, w]
  x: (32, 64, 2048) f32, weights: (2048, 64, 64, 3) f32, bias: (64, 2048) f32
  out: (32, 64, 2048) f32.  xp = x padded by 1 on both sides of the last axis.

Sharding: output_width (2048) split into 8 chunks of 256, one per core.

All operands are converted to bf16 on the host (norm rel-err lands ~2e-3,
well inside the 2e-2 gate).  That (a) halves HBM traffic -- the kernel is
memory-bound at ~8.4 MB/core over ~358 GB/s -- and (b) runs the PE at
1 cycle/row instead of 4.

Per position w the (ic,k)+bias contraction (193 rows) is two PSUM-accumulated
matmuls with the X PATCH STATIONARY (LDWEIGHTS cost scales with stationary
columns: 32 for the patch vs 64 for the weights):
  mm1: K=128 rows = (k=0, ic) ++ (k=1, ic), lhsT=x1[:,w,:] [128,32b],
       rhs=wa[:,w,:] [128,64oc]
  mm2: K=65 rows = (k=2, ic) ++ ones row,   lhsT=x2[:,w,:] [65,32b],
       rhs=wb[:,w,:] [65,64oc]  (bias folded in as wb row 64)
Output per position is [32b, 64oc] in PSUM.  Four consecutive positions are
packed across PSUM partitions via column tiling (psum base partition 32j ->
tile_position (0,32j)), and 8 such groups across the free dim, so one PSUM
bank holds 32 positions as [128, 512] and the Act-engine PSUM->SBUF copies
(with bf16 downcast) use all 128 lanes.

Host-side prep (numpy, cheap vs. the device HBM traffic):
  x1[j, c, b]  = xp[b, j%64, ws+c + j//64]      j in [0,128)
  x2[j, c, b]  = xp[b, j, ws+c+2] for j<64;  x2[64, c, b] = 1.0
  wa[j, c, oc] = W[ws+c, oc, j%64, j//64]       j in [0,128)
  wb[j, c, oc] = W[ws+c, oc, j, 2] for j<64; wb[64, c, oc] = bias[oc, ws+c]
  out_d[32*j+b, u, oc] -> out[b, oc, ws + 4*u + j]
"""

import numpy as np
from ml_dtypes import bfloat16

import concourse.bacc as bacc
import concourse.mybir as mybir
import concourse.tile as tile
from concourse.bass_utils import run_bass_kernel_spmd

B, IC, OC, KS, W = 32, 64, 64, 3, 2048
NCORES = 8
OWC = W // NCORES  # 256 positions per core
PG = 4             # positions packed across psum partitions (col tiling)
UG = 8             # position-groups packed across psum free dim
PSP = PG * UG      # 32 positions per psum bank tile
BF = mybir.dt.bfloat16
F32 = mybir.dt.float32

_compiled_nc = None


def _build_nc():
    nc = bacc.Bacc("TRN2")

    x1_d = nc.dram_tensor("x1", [2 * IC, OWC, B], BF, kind="ExternalInput")
    x2_d = nc.dram_tensor("x2", [IC + 1, OWC, B], BF, kind="ExternalInput")
    wa_d = nc.dram_tensor("wa", [2 * IC, OWC, OC], BF, kind="ExternalInput")
    wb_d = nc.dram_tensor("wb", [IC + 1, OWC, OC], BF, kind="ExternalInput")
    out_d = nc.dram_tensor("out", [PG * B, OWC // PG, OC], BF, kind="ExternalOutput")

    # First DMA slices are small so the PE starts quickly; the rest are fat.
    dma_slices = [(0, PSP), (PSP, PSP)]
    p = 2 * PSP
    while p < OWC:
        dma_slices.append((p, min(64, OWC - p)))
        p += 64

    with tile.TileContext(nc) as tc:
        with (
            tc.tile_pool(name="w", bufs=2) as wpool,
            tc.tile_pool(name="x", bufs=2) as xpool,
            tc.tile_pool(name="o", bufs=3) as opool,
            tc.tile_pool(name="ps", bufs=4, space="PSUM") as pspool,
        ):
            loaded = []  # (start, len, wa, wb, x1, x2)

            def load_slice(si):
                p0, plen = dma_slices[si]
                sl = slice(p0, p0 + plen)
                wa = wpool.tile([2 * IC, plen, OC], BF, tag="wa", name=f"wa_{si}")
                wb = wpool.tile([IC + 1, plen, OC], BF, tag="wb", name=f"wb_{si}")
                x1 = xpool.tile([2 * IC, plen, B], BF, tag="x1", name=f"x1_{si}")
                x2 = xpool.tile([IC + 1, plen, B], BF, tag="x2", name=f"x2_{si}")
                # split across the two HWDGE queues so issue/completion overlap
                nc.sync.dma_start(out=wa[:], in_=wa_d[:, sl, :])
                nc.sync.dma_start(out=x2[:], in_=x2_d[:, sl, :])
                nc.scalar.dma_start(out=x1[:], in_=x1_d[:, sl, :])
                nc.scalar.dma_start(out=wb[:], in_=wb_d[:, sl, :])
                loaded.append((p0, plen, wa, wb, x1, x2))

            load_slice(0)
            load_slice(1)
            for si in range(len(dma_slices)):
                if si >= 1 and si + 1 < len(dma_slices):
                    load_slice(si + 1)
                p0, plen, wa, wb, x1, x2 = loaded[si]
                for c0 in range(0, plen, PSP):
                    ps = pspool.tile([PG * B, UG, OC], F32, tag="ps",
                                     name=f"ps_{p0 + c0}")
                    for u in range(UG):
                        for j in range(PG):
                            wl = c0 + u * PG + j
                            pj = ps[j * B : (j + 1) * B, u, :]
                            nc.tensor.matmul(
                                pj, x1[:, wl, :], wa[:, wl, :],
                                start=True, stop=False,
                                tile_position=(0, j * B),
                            )
                            nc.tensor.matmul(
                                pj, x2[:, wl, :], wb[:, wl, :],
                                start=False, stop=True,
                                tile_position=(0, j * B),
                            )
                    ob = opool.tile([PG * B, UG, OC], BF, tag="ob",
                                    name=f"ob_{p0 + c0}")
                    nc.scalar.copy(out=ob[:], in_=ps[:])
                    u0 = (p0 + c0) // PG
                    nc.sync.dma_start(
                        out=out_d[:, u0 : u0 + UG, :], in_=ob[:]
                    )

    nc.compile()
    return nc


def _get_nc():
    global _compiled_nc
    if _compiled_nc is None:
        _compiled_nc = _build_nc()
    return _compiled_nc


def shard_inputs(x, weights, bias):
    x = np.ascontiguousarray(np.asarray(x, dtype=np.float32))
    weights = np.asarray(weights, dtype=np.float32)
    bias = np.asarray(bias, dtype=np.float32)

    xp = np.pad(x, ((0, 0), (0, 0), (1, 1)))
    xpT = np.ascontiguousarray(xp.transpose(1, 2, 0))  # (IC, W+2, B)
    ones = np.ones((1, OWC, B), np.float32)

    in_maps = []
    for c in range(NCORES):
        ws = c * OWC
        x1 = np.concatenate(
            [xpT[:, ws : ws + OWC, :], xpT[:, ws + 1 : ws + 1 + OWC, :]], axis=0
        )
        x2 = np.concatenate([xpT[:, ws + 2 : ws + 2 + OWC, :], ones], axis=0)
        wsl = weights[ws : ws + OWC]  # (OWC, OC, IC, KS)
        wa = wsl[:, :, :, 0:2].transpose(3, 2, 0, 1).reshape(2 * IC, OWC, OC)
        wb = np.concatenate(
            [wsl[:, :, :, 2].transpose(2, 0, 1), bias[:, ws : ws + OWC].T[None]],
            axis=0,
        )
        in_maps.append(
            {
                "x1": np.ascontiguousarray(x1.astype(bfloat16)),
                "x2": np.ascontiguousarray(x2.astype(bfloat16)),
                "wa": np.ascontiguousarray(wa.astype(bfloat16)),
                "wb": np.ascontiguousarray(wb.astype(bfloat16)),
            }
        )
    return in_maps


def run_sharded(x, weights, bias, trace=False):
    nc = _get_nc()
    in_maps = shard_inputs(x, weights, bias)
    res = run_bass_kernel_spmd(nc, in_maps, list(range(NCORES)), trace=trace)
    out = np.empty((B, OC, W), np.float32)
    for c in range(NCORES):
        oc_arr = res.results[c]["out"].astype(np.float32)  # [128, 64, 64]
        oc_arr = oc_arr.reshape(PG, B, OWC // PG, OC)      # [j, b, u, oc]
        out[:, :, c * OWC : (c + 1) * OWC] = (
            oc_arr.transpose(1, 3, 2, 0).reshape(B, OC, OWC)
        )
    return out, res


def kernel(x, weights, bias):
    out, _ = run_sharded(x, weights, bias)
    return out


# revision 4
# speedup vs baseline: 1.0635x; 1.0635x over previous
"""LocallyConnected1d Trainium2 kernel (bf16).

Problem: out[b, oc, w] = sum_{ic,k} xp[b, ic, w+k] * W[w, oc, ic, k] + bias[oc, w]
  x: (32, 64, 2048) f32, weights: (2048, 64, 64, 3) f32, bias: (64, 2048) f32
  out: (32, 64, 2048) f32.  xp = x padded by 1 on both sides of the last axis.

Sharding: output_width (2048) split into 8 chunks of 256, one per core.

All operands are converted to bf16 on the host (norm rel-err lands ~3e-3,
inside the 2e-2 gate).  That (a) halves HBM traffic -- the kernel is
memory-bound at ~10.5 MB/core over ~358 GB/s/core -- and (b) runs the PE
at 1 cycle/row instead of 4.

Per position w the (ic,k)+bias contraction (193 rows) is two PSUM-accumulated
matmuls with the X PATCH STATIONARY (LDWEIGHTS cost scales with stationary
columns: 32 for the patch vs 64 for the weights):
  mm1: K=128 rows = (k=0, ic) ++ (k=1, ic), lhsT=x1[:,w,:] [128,32b],
       rhs=wa[:,w,:] [128,64oc]
  mm2: K=65 rows = (k=2, ic) ++ ones row,   lhsT=x2[:,w,:] [65,32b],
       rhs=wb[:,w,:] [65,64oc]  (bias folded in as wb row 64)
Output per position is [32b, 64oc] in PSUM.  Four consecutive positions are
packed across PSUM partitions via column tiling (tile_position (0,32j)),
and 8 such groups across the free dim, so one PSUM bank holds 32 positions
as [128, 512].

Scheduling (the fp32->bf16 switch makes this DMA-bound, so the DMA queues
must never stall):
  - ALL input slices are enqueued up-front on the two HWDGE queues
    (everything fits in SBUF), byte-balanced: sync gets wa, scalar gets
    x1+wb+x2.  No pool recycling, so no input dma_start ever waits on a
    compute semaphore.
  - PSUM->SBUF copies run on DVE (tensor_copy), keeping the scalar queue
    free of the ACT_TABLE_LOAD it would otherwise pay.
  - Output DMAs sit on sync AFTER all input descriptor-gen in program
    order; when they block on a copy semaphore nothing is behind them.
  - First/last slices are 32 positions (vs 64) to shorten pipeline ramp
    and drain.

Host-side prep (numpy, cheap vs. the device HBM traffic):
  x1[j, c, b]  = xp[b, j%64, ws+c + j//64]      j in [0,128)
  x2[j, c, b]  = xp[b, j, ws+c+2] for j<64;  x2[64, c, b] = 1.0
  wa[j, c, oc] = W[ws+c, oc, j%64, j//64]       j in [0,128)
  wb[j, c, oc] = W[ws+c, oc, j, 2] for j<64; wb[64, c, oc] = bias[oc, ws+c]
  out_d[32*j+b, u, oc] -> out[b, oc, ws + 4*u + j]
"""

import numpy as np
from ml_dtypes import bfloat16

import concourse.bacc as bacc
import concourse.mybir as mybir
import concourse.tile as tile
from concourse.bass_utils import run_bass_kernel_spmd

B, IC, OC, KS, W = 32, 64, 64, 3, 2048
NCORES = 8
OWC = W // NCORES  # 256 positions per core
PG = 4             # positions packed across psum partitions (col tiling)
UG = 8             # position-groups packed across psum free dim
PSP = PG * UG      # 32 positions per psum bank tile
BF = mybir.dt.bfloat16
F32 = mybir.dt.float32

DMA_SLICES = [(0, 32), (32, 64), (96, 64), (160, 64), (224, 32)]

_compiled_nc = None


def _build_nc():
    nc = bacc.Bacc("TRN2")

    x1_d = nc.dram_tensor("x1", [2 * IC, OWC, B], BF, kind="ExternalInput")
    x2_d = nc.dram_tensor("x2", [IC + 1, OWC, B], BF, kind="ExternalInput")
    wa_d = nc.dram_tensor("wa", [2 * IC, OWC, OC], BF, kind="ExternalInput")
    wb_d = nc.dram_tensor("wb", [IC + 1, OWC, OC], BF, kind="ExternalInput")
    out_d = nc.dram_tensor("out", [PG * B, OWC // PG, OC], BF, kind="ExternalOutput")

    with tile.TileContext(nc) as tc:
        with (
            tc.tile_pool(name="w", bufs=1) as wpool,
            tc.tile_pool(name="x", bufs=1) as xpool,
            tc.tile_pool(name="o", bufs=3) as opool,
            tc.tile_pool(name="ps", bufs=4, space="PSUM") as pspool,
        ):
            loaded = []
            # every input slice enqueued up-front; nothing on these queues
            # ever waits on compute
            for si, (p0, plen) in enumerate(DMA_SLICES):
                sl = slice(p0, p0 + plen)
                wa = wpool.tile([2 * IC, plen, OC], BF, tag=f"wa{si}", name=f"wa_{si}")
                wb = wpool.tile([IC + 1, plen, OC], BF, tag=f"wb{si}", name=f"wb_{si}")
                x1 = xpool.tile([2 * IC, plen, B], BF, tag=f"x1{si}", name=f"x1_{si}")
                x2 = xpool.tile([IC + 1, plen, B], BF, tag=f"x2{si}", name=f"x2_{si}")
                nc.sync.dma_start(out=wa[:], in_=wa_d[:, sl, :])
                nc.scalar.dma_start(out=x1[:], in_=x1_d[:, sl, :])
                nc.scalar.dma_start(out=wb[:], in_=wb_d[:, sl, :])
                nc.scalar.dma_start(out=x2[:], in_=x2_d[:, sl, :])
                loaded.append((p0, plen, wa, wb, x1, x2))

            for p0, plen, wa, wb, x1, x2 in loaded:
                for c0 in range(0, plen, PSP):
                    ps = pspool.tile([PG * B, UG, OC], F32, tag="ps",
                                     name=f"ps_{p0 + c0}")
                    for u in range(UG):
                        for j in range(PG):
                            wl = c0 + u * PG + j
                            pj = ps[j * B : (j + 1) * B, u, :]
                            nc.tensor.matmul(
                                pj, x1[:, wl, :], wa[:, wl, :],
                                start=True, stop=False,
                                tile_position=(0, j * B),
                            )
                            nc.tensor.matmul(
                                pj, x2[:, wl, :], wb[:, wl, :],
                                start=False, stop=True,
                                tile_position=(0, j * B),
                            )
                    ob = opool.tile([PG * B, UG, OC], BF, tag="ob",
                                    name=f"ob_{p0 + c0}")
                    nc.vector.tensor_copy(out=ob[:], in_=ps[:])
                    u0 = (p0 + c0) // PG
                    nc.sync.dma_start(
                        out=out_d[:, u0 : u0 + UG, :], in_=ob[:]
                    )

    nc.compile()
    return nc


def _get_nc():
    global _compiled_nc
    if _compiled_nc is None:
        _compiled_nc = _build_nc()
    return _compiled_nc


def shard_inputs(x, weights, bias):
    x = np.ascontiguousarray(np.asarray(x, dtype=np.float32))
    weights = np.asarray(weights, dtype=np.float32)
    bias = np.asarray(bias, dtype=np.float32)

    xp = np.pad(x, ((0, 0), (0, 0), (1, 1)))
    xpT = np.ascontiguousarray(xp.transpose(1, 2, 0))  # (IC, W+2, B)
    ones = np.ones((1, OWC, B), np.float32)

    in_maps = []
    for c in range(NCORES):
        ws = c * OWC
        x1 = np.concatenate(
            [xpT[:, ws : ws + OWC, :], xpT[:, ws + 1 : ws + 1 + OWC, :]], axis=0
        )
        x2 = np.concatenate([xpT[:, ws + 2 : ws + 2 + OWC, :], ones], axis=0)
        wsl = weights[ws : ws + OWC]  # (OWC, OC, IC, KS)
        wa = wsl[:, :, :, 0:2].transpose(3, 2, 0, 1).reshape(2 * IC, OWC, OC)
        wb = np.concatenate(
            [wsl[:, :, :, 2].transpose(2, 0, 1), bias[:, ws : ws + OWC].T[None]],
            axis=0,
        )
        in_maps.append(
            {
                "x1": np.ascontiguousarray(x1.astype(bfloat16)),
                "x2": np.ascontiguousarray(x2.astype(bfloat16)),
                "wa": np.ascontiguousarray(wa.astype(bfloat16)),
                "wb": np.ascontiguousarray(wb.astype(bfloat16)),
            }
        )
    return in_maps


def run_sharded(x, weights, bias, trace=False):
    nc = _get_nc()
    in_maps = shard_inputs(x, weights, bias)
    res = run_bass_kernel_spmd(nc, in_maps, list(range(NCORES)), trace=trace)
    out = np.empty((B, OC, W), np.float32)
    for c in range(NCORES):
        oc_arr = res.results[c]["out"].astype(np.float32)  # [128, 64, 64]
        oc_arr = oc_arr.reshape(PG, B, OWC // PG, OC)      # [j, b, u, oc]
        out[:, :, c * OWC : (c + 1) * OWC] = (
            oc_arr.transpose(1, 3, 2, 0).reshape(B, OC, OWC)
        )
    return out, res


def kernel(x, weights, bias):
    out, _ = run_sharded(x, weights, bias)
    return out
